# revision 6
# baseline (speedup 1.0000x reference)
"""Trainium2 Bass kernel: per-voxel eigenvalues of 3x3 symmetric matrices.

Pair-packed redesign: each 32-bit SBUF word holds two bf16 channels of one
voxel ("pair tiles").  Hand-authored 2X_1PORT custom-DVE uop programs see
all four 16-bit lanes of their two operand streams per cycle and run an
arbitrary <=8-op DAG over them, writing a packed 2-channel result: one
32-bit pair per cycle per partition.  That collapses the 24-pass stock
tensor_tensor pipeline (1 ALU op / 2 elems / cycle) into 8 pair passes
(up to 8 ALU ops / pair / cycle).  ACT reads/writes pair halves via
stride-2 APs (ACT rate is stride/dtype-independent, ~+300ns for strided).

Math (trace-shifted, a+b+c=0; u,v,w true off-diagonals; q = tr/3):
    p2h = c^2 - ab + u^2+v^2+w^2
    det = abc + 2uvw - a w^2 - b v^2 - c u^2
    rr  = clamp(det * e1), e1 = exp(-1.5 ln p2h + B1); P2 = exp(.5 ln p2h + B2)
    at  = arctan(rr / sqrt(1-rr^2)) = asin(rr)
    m5  = P2 sin(-at/3 + 2pi/3),  m7 = P2 sin(at/3)
    lmax = q + m5; lmid = q - m7; lmin = q - m5 + m7   (host-side linear add,
    mirroring the host-side trace-shift prep; device computes m5, m7)

Pair passes ([128, 1728-pair] tiles, ~1.95us each):
    A (ab|cu): s2 = u^2-ab,        z  = abc - c u^2
    B (vw|ab): sv = v^2+w^2,       dd = a w^2 + b v^2
    C (vw|cu): m2 = 2uvw,          c2 = c^2
    D (A |C ): p1 = s2+c2,         z2 = z+m2
    E (D |B ): p2h = p1+sv,        det = z2-dd
    F (E'|- ): rr = clamp(det*e1), r2 = rr^2     (e1 ACT-written into E.lo)
    G (F'|- ): t2 = rr*invs, pad   (invs ACT-written into F.hi)
    Y (H |P ): m5 = P2*c1,         m7 = P2*s3
ACT (8/rep): Ln, Exp e1, Exp P2 | Ln 1-r2, Exp invs | Arctan, Sin, Sin.

Emission is software-pipelined at depth 4 (stages S0=loads+A..E+lnexp1,
S1=F+lnexp2, S2=G, S2b=trig batch x2 reps, S3=Y+store): engine queues are
in-order, so V's queue must not head-block on ACT mid-rep.  Trig ACT ops
of two reps are emitted adjacently to reduce table-set switches.

Measured (NTFF, in-NEFF unroll differencing, (T(7)-T(1))/6): 19.8us/iter
steady state (f32 stock baseline: 93.5us; bf16 stock-op baseline: 23.9us).
Engine budgets per iter: Vector 8 pair passes x ~1.95us = 15.6us (~76%
busy); ACT 8 ops x ~2.0us + ~2.9 table loads x 1.28us = 19.8us (~100%
busy, the bottleneck).  ACT strided *writes* cost +470ns each; strided
reads are free.  Remaining known headroom: walrus lower_act picks
per-function table sets greedily (Ln->natural_log, Exp->exp_and_others)
so steady state pays ~2.9 loads/iter instead of 1 (~2.4us); pre-placing
InstLoadActFuncSet broke numerics on HW and was reverted.
"""

import sys

if "/opt/trn_rl_repo" not in sys.path:
    sys.path.insert(0, "/opt/trn_rl_repo")

import math

import numpy as np


# ===== inlined pair-op machinery (hand-authored 2X_1PORT custom DVE) =====
from dataclasses import dataclass

from concourse.dve_uop import (
    DISABLE,
    ENABLE,
    AluInp,
    AluOp,
    DelayInp,
    DveOpSpec,
    InpSel,
    OutPath,
    OutSel,
    Trigger,
    UopConfig,
    UopDpConfig,
    DELAY_OUT,
)

OP_MAP = {
    "add": AluOp.ADD,
    "sub": AluOp.SUBTRACT,
    "mul": AluOp.MULTIPLY,
    "max": AluOp.MAX,
    "min": AluOp.MIN,
}

LEAF_SEL = {
    "a": InpSel.SRC_0,
    "ah": InpSel.SRC_0_HI,
    "b": InpSel.SRC_1,
    "bh": InpSel.SRC_1_HI,
    "c0": InpSel.CONST_0,
    "c1": InpSel.CONST_1,
    "c2": InpSel.CONST_2,
    "one": InpSel.ONE_F32,
    "zero": InpSel.ZERO,
}

N_BLOCKS = 8
N_CHAINS = 6


def build_pair_uop(ssa, outs):
    """Schedule `ssa` (one op per block, in order) into a UopConfig."""
    n = len(ssa)
    assert n <= N_BLOCKS, f"{n} ops > {N_BLOCKS} blocks"
    dests = [d for d, *_ in ssa]
    assert len(set(dests)) == n, "SSA dests must be unique"

    # last block that reads each value / leaf; outputs read at block 8
    last_read = {}
    for k, (_, _, sa, sb) in enumerate(ssa):
        for s in (sa, sb):
            last_read[s] = k
    born = {d: k for k, (d, *_r) in enumerate(ssa)}
    for o in outs:
        assert o in born, f"output {o} not produced"
        last_read[o] = N_BLOCKS

    # leaves used, in first-use order
    leaves = []
    for _, _, sa, sb in ssa:
        for s in (sa, sb):
            if s in LEAF_SEL and s not in leaves:
                leaves.append(s)
    assert len(leaves) <= N_CHAINS, f"too many leaves: {leaves}"

    u = UopConfig()
    u.trigger = (Trigger.SRC_TENSOR_DONE, Trigger.NONE, Trigger.NONE)
    u.next_uop = (0, 0, 0)
    u.require_inp0 = ENABLE
    uses_src1 = any(s in ("b", "bh") for s in leaves)
    u.require_inp1 = ENABLE if uses_src1 else DISABLE

    # chain c <- inp lane c+1
    loc = {}          # value/leaf -> ('chain', c) or ('alu', born_block)
    chain_free = list(range(N_CHAINS))
    chain_owner = {}  # c -> value (for pass-through bookkeeping)
    for lf in leaves:
        c = chain_free.pop(0)
        u.enable_input(LEAF_SEL[lf], c + 1)
        loc[lf] = ("chain", c)
        chain_owner[c] = lf

    # the single value allowed to ride the tail ALU-bypass chain to blk7:
    # an output born at the last block
    alu_rider = None
    for o in outs:
        if born[o] == n - 1:
            alu_rider = o
            break

    val_chain = {}  # value -> chain id (captured at born+1, visible born+2)

    def op_src(s, k):
        where = loc[s]
        if where[0] == "chain":
            return AluInp(AluInp.PREV_DELAY_0 + where[1])
        if where == ("alu", k - 1):
            return AluInp.PREV_ALU_OUT
        if s in val_chain and k >= where[1] + 2:
            return AluInp(AluInp.PREV_DELAY_0 + val_chain[s])
        raise AssertionError(
            f"value {s} born block {where[1]} not reachable at block {k}"
        )

    pending_capture = None  # (value, chain) to capture at current block
    for k, (d, opn, sa, sb) in enumerate(ssa):
        blk = u.datapath_config[k]
        blk.enable_alu(OP_MAP[opn], op_src(sa, k), op_src(sb, k))

        # pass-through all live chains; apply pending capture
        for c, owner in list(chain_owner.items()):
            if pending_capture and pending_capture[1] == c:
                continue
            if last_read.get(owner, -1) > k:
                blk.pass_through_delay(c)
            else:
                del chain_owner[c]
                chain_free.append(c)
                chain_free.sort()
        if pending_capture:
            v, c = pending_capture
            blk.enable_delay_from_src(DelayInp.PREV_ALU_OUT, c)
            pending_capture = None

        # where does d live?
        needs_chain = (
            last_read.get(d, -1) > k + 1
            or (d in outs and d is not alu_rider)
            or (last_read.get(d, -1) == k + 1 and d in outs and d is not alu_rider)
        )
        loc[d] = ("alu", k)
        if needs_chain:
            assert chain_free, f"no free delay chain for {d} at block {k}"
            c = chain_free.pop(0)
            chain_owner[c] = d
            pending_capture = (d, c)
            val_chain[d] = c
            # consumers at k+1 still read PREV_ALU_OUT; from k+2 on, the
            # chain (op_src falls back through val_chain).

    # tail blocks: bypass ALU (carries last op's result), pass chains through
    for k in range(n, N_BLOCKS):
        blk = u.datapath_config[k]
        blk.pass_through_alu()
        for c, owner in list(chain_owner.items()):
            if pending_capture and pending_capture[1] == c:
                continue
            if last_read.get(owner, -1) > k:
                blk.pass_through_delay(c)
        if pending_capture:
            v, c = pending_capture
            blk.enable_delay_from_src(DelayInp.PREV_ALU_OUT, c)
            pending_capture = None
        # values captured become chain-resident
        for cc, owner in chain_owner.items():
            loc[owner] = ("chain", cc)

    # resolve chain locations for captured values (loc may still say 'alu')
    for c, owner in chain_owner.items():
        loc[owner] = ("chain", c)

    # outputs
    def out_sel(o):
        if o is alu_rider:
            return OutSel.ALU_OUT
        where = loc[o]
        assert where[0] == "chain", f"output {o} lost: {where}"
        return DELAY_OUT[where[1]]

    if len(outs) == 2:
        u.enable_output(out_sel(outs[0]), OutPath.WR0_LO)
        u.enable_output(out_sel(outs[1]), OutPath.WR0_HI)
    else:
        u.enable_output(out_sel(outs[0]), OutPath.WR0_LO)
    u.validate("v3")
    return u


@dataclass
class PairOp:
    name: str
    ssa: list
    outs: tuple
    uses_src1: bool


_REGISTERED = {}


def register_pair_op(name, ssa, outs):
    """Build + register a pair op under dve_ops so the per-NEFF table
    includes it. Returns the DveOp for _custom_dve."""
    from concourse import dve_ops
    from concourse.dve_spec import Spec, Src0, Src1, C0, C1, C2, sq

    if name in _REGISTERED:
        return _REGISTERED[name]

    uop = build_pair_uop(ssa, outs)
    leaves = {s for _, _, sa, sb in ssa for s in (sa, sb)}
    # ALWAYS two-stream: a single-src packed-bf16 op would be auto-promoted
    # to 2x_2p/4x mode slots whose data rates don't match our 2x program
    # (observed NRT_EXEC_UNIT_UNRECOVERABLE). With rd1_en the engine treats
    # it as TT-class and caps detection at 2X_1PORT. Callers pass a dummy
    # in1 when the program doesn't read b/bh.
    uses_src1 = True
    uop.require_inp1 = ENABLE

    # dummy Spec carrying the right leaf set for _custom_dve's asserts
    body = Src0 + Src0
    if uses_src1:
        body = body + Src1
    if "c0" in leaves:
        body = body * C0
    if "c1" in leaves:
        body = body * C1
    if "c2" in leaves:
        body = body * C2
    spec = Spec(body=body, reference=lambda *a, **k: None)

    if name in dve_ops._SUB_OPCODE_FOR_NAME:
        row = dve_ops._SUB_OPCODE_FOR_NAME[name]
        op = dve_ops.DveOp(name, spec, subdim=False, uops_sha={})
        dve_ops.OPS[:] = [o for o in dve_ops.OPS if o.name != name] + [op]
    else:
        row = max(dve_ops._SUB_OPCODE_FOR_NAME.values()) + 1
        assert row < 0x20, "byte-36 row overflow"
        op = dve_ops.DveOp(name, spec, subdim=False, uops_sha={})
        dve_ops.OPS.append(op)
        dve_ops._SUB_OPCODE_FOR_NAME[name] = row
    dve_ops.CUSTOM_DVE_SPECS[name] = spec

    compiled = DveOpSpec(
        name=name,
        opcode=row,
        uops=[uop],      # 1x slot: same program; 1x must never engage
        uops_2x=[uop],
        perf_max=1,
        rd1_en=uses_src1,
    )
    dve_ops._COMPILE_CACHE[(name, "v3")] = compiled
    _REGISTERED[name] = op
    return op


def emit(nc, op, out, in0, in1=None, s0=0.0, s1=0.0, imm2=0.0):
    """Emit one pair-op instruction with perf_max=1 (2x_1p reachable)."""
    for ap in (out, in0) + ((in1,) if in1 is not None else ()):
        pass
    bi = nc.vector._custom_dve(
        op, out=out, in0=in0, in1=in1, s0=s0, s1=s1, imm2=imm2
    )
    bi.ins.perf_max = 1
    return bi


def eval_ssa(ssa, outs, env):
    """numpy reference for a pair program; env maps leaves to arrays."""
    import numpy as np

    vals = dict(env)
    vals.setdefault("one", 1.0)
    vals.setdefault("zero", 0.0)
    fns = {
        "add": lambda x, y: x + y,
        "sub": lambda x, y: x - y,
        "mul": lambda x, y: x * y,
        "max": lambda x, y: __import__("numpy").maximum(x, y),
        "min": lambda x, y: __import__("numpy").minimum(x, y),
    }
    for d, opn, sa, sb in ssa:
        vals[d] = fns[opn](vals[sa], vals[sb])
    return [vals[o] for o in outs]


# ===== kernel =====


N_CORES = 8
B = 2
DHW = 96 * 96 * 96
PER = DHW // N_CORES        # 110592 voxels per batch per core
P = 128
FB = PER // P               # 864
NP = B * FB                 # 1728 voxel-pairs per partition per core
FD = 2 * NP                 # 3456 bf16 elems per pair tile row

EPS_P2 = 1e-8
R2C = 1.0 - 2.0 ** -8
CL = math.sqrt(R2C)
B1 = 1.5 * math.log(6.0) - 2.5 * math.log(2.0)
B2 = 1.5 * math.log(2.0) - 0.5 * math.log(6.0)
TWO_PI_3 = 2.0 * math.pi / 3.0

_CACHE = {}

OPA = register_pair_op(
    "EIGA",
    [
        ("m3", "mul", "a", "ah"),
        ("m4", "mul", "m3", "b"),
        ("u2", "mul", "bh", "bh"),
        ("d3", "mul", "u2", "b"),
        ("z", "sub", "m4", "d3"),
        ("s2", "sub", "u2", "m3"),
    ],
    ("s2", "z"),
)
OPB = register_pair_op(
    "EIGB",
    [
        ("v2", "mul", "a", "a"),
        ("w2", "mul", "ah", "ah"),
        ("sv", "add", "v2", "w2"),
        ("d1", "mul", "b", "w2"),
        ("d2", "mul", "bh", "v2"),
        ("dd", "add", "d1", "d2"),
    ],
    ("sv", "dd"),
)
OPC = register_pair_op(
    "EIGC",
    [
        ("mv", "mul", "a", "ah"),
        ("mu", "mul", "mv", "bh"),
        ("m2", "mul", "mu", "c0"),
        ("c2", "mul", "b", "b"),
    ],
    ("m2", "c2"),
)
OPD = register_pair_op(
    "EIGD",
    [("p1", "add", "a", "bh"), ("z2", "add", "ah", "b")],
    ("p1", "z2"),
)
OPE = register_pair_op(
    "EIGE",
    [("p2h", "add", "a", "b"), ("det", "sub", "ah", "bh")],
    ("p2h", "det"),
)
OPF = register_pair_op(
    "EIGF",
    [
        ("t", "mul", "a", "ah"),
        ("tc", "min", "t", "c0"),
        ("rr", "max", "tc", "c1"),
        ("r2", "mul", "rr", "rr"),
    ],
    ("rr", "r2"),
)
OPG = register_pair_op(
    "EIGG",
    [("t2", "mul", "a", "ah"), ("j", "add", "t2", "zero")],
    ("t2", "j"),
)
OPY = register_pair_op(
    "EIGYM",
    [("m5", "mul", "bh", "a"), ("m7", "mul", "bh", "ah")],
    ("m5", "m7"),
)


def _build(nrep=1, split_waits=True):
    import concourse.bass as bass
    import concourse.tile as tile
    from concourse import mybir

    fp16 = mybir.dt.bfloat16
    fp32 = mybir.dt.float32
    AF = mybir.ActivationFunctionType

    nc = bass.Bass("TRN2", target_bir_lowering=False, debug=False,
                   num_devices=N_CORES)
    x = nc.dram_tensor("x", [3, P, FD], fp16, kind="ExternalInput").ap()
    y = nc.dram_tensor("y", [1, P, FD], fp16, kind="ExternalOutput").ap()

    for cval in (B1, B2, TWO_PI_3, EPS_P2, 1.0):
        cval = float(cval)
        if (fp32, cval) not in nc.const_aps.aps:
            ctens = nc.alloc_sbuf_tensor(f"const-f32-{cval}", [128, 1], fp32)
            nc.gpsimd.memset(ctens.ap(), cval)
            nc.const_aps.aps[(fp32, cval)] = ctens.ap()
    nc.all_engine_barrier()

    S = nc.scalar
    NPAR = 4
    # slot aliasing (live ranges don't overlap within a parity):
    SLOT = {"pab": 0, "E": 0, "pcu": 1, "F": 1, "pvw": 2, "D": 2, "H": 2,
            "C": 3, "tP": 3, "A": 4, "Y": 4, "B": 5, "G": 5,
            "LN": 6, "LM": 6, "AT": 6}

    with tile.TileContext(nc) as tc:
        with tc.tile_pool(name="eig", bufs=1) as pool:
            tiles = [{} for _ in range(nrep)]

            def T(rep, name, dtype=fp16, w=FD):
                par = rep % NPAR
                t = pool.tile([P, w], dtype, tag=f"s{SLOT[name]}p{par}",
                              name=f"{name}r{rep}")
                tiles[rep][name] = t
                return t

            def stage0(r):
                pab, pcu, pvw = (T(r, n) for n in ("pab", "pcu", "pvw"))
                for t, ch in ((pab, 0), (pcu, 1), (pvw, 2)):
                    nc.sync.dma_start(out=t[:, :], in_=x[ch][:, :])
                tA, tB, tC = T(r, "A"), T(r, "B"), T(r, "C")
                emit(nc, OPA, out=tA[:, :], in0=pab[:, :],
                             in1=pcu[:, :])
                emit(nc, OPB, out=tB[:, :], in0=pvw[:, :],
                             in1=pab[:, :])
                emit(nc, OPC, out=tC[:, :], in0=pvw[:, :],
                             in1=pcu[:, :], s0=2.0)
                tD, tE = T(r, "D"), T(r, "E")
                emit(nc, OPD, out=tD[:, :], in0=tA[:, :],
                             in1=tC[:, :])
                emit(nc, OPE, out=tE[:, :], in0=tD[:, :],
                             in1=tB[:, :])
                LN = T(r, "LN", fp32, NP)
                S.activation(LN[:, :], tE[:, 0::2], AF.Ln, bias=float(EPS_P2))
                S.activation(tE[:, 0::2], LN[:, :], AF.Exp, scale=-1.5,
                             bias=float(B1))
                tP = T(r, "tP")
                S.activation(tP[:, 1::2], LN[:, :], AF.Exp, scale=0.5,
                             bias=float(B2))

            def stage1(r):
                tE = tiles[r]["E"]
                tF = T(r, "F")
                emit(nc, OPF, out=tF[:, :], in0=tE[:, :],
                             in1=tE[:, :], s0=CL, s1=-CL)
                LM = T(r, "LM", fp32, NP)
                S.activation(LM[:, :], tF[:, 1::2], AF.Ln, scale=-1.0,
                             bias=1.0)
                S.activation(tF[:, 1::2], LM[:, :], AF.Exp, scale=-0.5)

            def stage2v(r):
                tF = tiles[r]["F"]
                tG = T(r, "G")
                emit(nc, OPG, out=tG[:, :], in0=tF[:, :],
                             in1=tF[:, :])

            def stage2a(r):
                tG = tiles[r]["G"]
                AT = T(r, "AT", fp32, NP)
                S.activation(AT[:, :], tG[:, 0::2], AF.Arctan)
                tH = T(r, "H")
                S.activation(tH[:, 0::2], AT[:, :], AF.Sin, scale=-1.0 / 3.0,
                             bias=float(TWO_PI_3))
                S.activation(tH[:, 1::2], AT[:, :], AF.Sin, scale=1.0 / 3.0)

            def stage3(r):
                tH, tP = tiles[r]["H"], tiles[r]["tP"]
                tY = T(r, "Y")
                emit(nc, OPY, out=tY[:, :], in0=tH[:, :],
                             in1=tP[:, :])
                nc.sync.dma_start(out=y[0][:, :], in_=tY[:, :])

            for it in range(nrep + 4):
                if it < nrep:
                    stage0(it)
                if 0 <= it - 1 < nrep:
                    stage1(it - 1)
                if 0 <= it - 2 < nrep:
                    stage2v(it - 2)
                # batch two reps' trig ACT ops adjacently: one table-set
                # switch per rep instead of two.
                if it % 2 == 1 or it == nrep + 3:
                    for j in (it - 3, it - 2):
                        if 0 <= j < nrep and "AT" not in tiles[j]:
                            stage2a(j)
                    for j in (it - 3, it - 2):
                        if 0 <= j < nrep and "Y" not in tiles[j]:
                            stage3(j)

    from concourse.library_overlay import lower_extended_insts
    lower_extended_insts(nc)
    if split_waits:
        _split_multi_waits(nc, mybir)
    return nc


def _split_multi_waits(nc, mybir):
    """walrus codegen allows a single sync-wait slot per TPB instruction;
    hoist extra waits onto standalone NoOps on the same engine."""
    for f in nc.m.functions:
        for blk in f.blocks:
            il = blk.instructions
            i = 0
            while i < len(il):
                inst = il[i]
                si = inst.sync_info
                if si is not None and si.on_wait and len(si.on_wait) > 1:
                    waits = list(si.on_wait)
                    for w in waits[:-1]:
                        nop = mybir.InstNoOp(
                            name=nc.get_next_instruction_name(),
                            engine=inst.engine,
                            ins=[],
                            outs=[],
                            sync_info=mybir.SyncInfo(on_wait=[w], on_update=[]),
                            bass_nofuse=True,
                        )
                        il.insert(i, nop)
                        i += 1
                    si.on_wait = waits[-1:]
                i += 1


def get_program():
    if "nc" not in _CACHE:
        _CACHE["nc"] = _build()
    return _CACHE["nc"]


def shard_inputs(X):
    """X: (2,9,96,96,96) f32 -> per-core {"x": (3,P,FD) bf16} pair channels
    (a,b), (c,u), (v,w), plus host-side q per core for unshard."""
    import ml_dtypes

    x = np.asarray(X, dtype=np.float32).reshape(B, 9, DHW)
    q = (x[:, 0] + x[:, 4] + x[:, 8]) * (1.0 / 3.0)
    ch = np.empty((6, B, DHW), dtype=np.float32)
    ch[0] = x[:, 0] - q
    ch[1] = x[:, 4] - q
    ch[2] = x[:, 8] - q
    ch[3] = (x[:, 1] + x[:, 3]) * 0.5
    ch[4] = (x[:, 2] + x[:, 6]) * 0.5
    ch[5] = (x[:, 5] + x[:, 7]) * 0.5
    pairs = [(0, 1), (2, 3), (4, 5)]
    maps = []
    for c in range(N_CORES):
        slab = ch[:, :, c * PER:(c + 1) * PER].reshape(6, B, P, FB)
        slab = np.ascontiguousarray(slab.transpose(0, 2, 1, 3)).reshape(6, P, NP)
        xc = np.empty((3, P, FD), dtype=ml_dtypes.bfloat16)
        for k, (lo, hi) in enumerate(pairs):
            xc[k, :, 0::2] = slab[lo]
            xc[k, :, 1::2] = slab[hi]
        maps.append({"x": xc})
    return maps, q


def unshard_outputs(results, q):
    out = np.empty((B, 3, DHW), dtype=np.float32)
    for c, r in enumerate(results):
        yc = np.asarray(r["y"]).astype(np.float32)   # (1, P, FD)
        m5 = yc[0, :, 0::2]
        m7 = yc[0, :, 1::2]
        # (P, NP) -> (B, PER)
        m5 = m5.reshape(P, B, FB).transpose(1, 0, 2).reshape(B, PER)
        m7 = m7.reshape(P, B, FB).transpose(1, 0, 2).reshape(B, PER)
        qc = q[:, c * PER:(c + 1) * PER]
        out[:, 0, c * PER:(c + 1) * PER] = qc - m5 + m7   # lmin
        out[:, 1, c * PER:(c + 1) * PER] = qc - m7        # lmid
        out[:, 2, c * PER:(c + 1) * PER] = qc + m5        # lmax
    return out.reshape(B, 3, 96, 96, 96)


def kernel(X):
    from concourse.bass_utils import run_bass_kernel_spmd

    nc = get_program()
    in_maps, q = shard_inputs(np.asarray(X))
    res = run_bass_kernel_spmd(nc, in_maps, list(range(N_CORES)))
    return unshard_outputs(res.results, q)


# revision 8
# speedup vs baseline: 1.1253x; 1.1253x over previous
"""Trainium2 Bass kernel: per-voxel eigenvalues of 3x3 symmetric matrices.

Pair-packed redesign: each 32-bit SBUF word holds two bf16 channels of one
voxel ("pair tiles").  Hand-authored 2X_1PORT custom-DVE uop programs see
all four 16-bit lanes of their two operand streams per cycle and run an
arbitrary <=8-op DAG over them, writing a packed 2-channel result: one
32-bit pair per cycle per partition.  That collapses the 24-pass stock
tensor_tensor pipeline (1 ALU op / 2 elems / cycle) into 8 pair passes
(up to 8 ALU ops / pair / cycle).  ACT reads/writes pair halves via
stride-2 APs (ACT rate is stride/dtype-independent, ~+300ns for strided).

Math (trace-shifted, a+b+c=0; u,v,w true off-diagonals; q = tr/3):
    p2h = c^2 - ab + u^2+v^2+w^2
    det = abc + 2uvw - a w^2 - b v^2 - c u^2
    rr  = clamp(det * e1), e1 = exp(-1.5 ln p2h + B1); P2 = exp(.5 ln p2h + B2)
    at  = arctan(rr / sqrt(1-rr^2)) = asin(rr)
    m5  = P2 sin(-at/3 + 2pi/3),  m7 = P2 sin(at/3)
    lmax = q + m5; lmid = q - m7; lmin = q - m5 + m7   (host-side linear add,
    mirroring the host-side trace-shift prep; device computes m5, m7)

Pair passes ([128, 1728-pair] tiles, ~1.95us each):
    A (ab|cu): s2 = u^2-ab,        z  = abc - c u^2
    B (vw|ab): sv = v^2+w^2,       dd = a w^2 + b v^2
    C (vw|cu): m2 = 2uvw,          c2 = c^2
    D (A |C ): p1 = s2+c2,         z2 = z+m2
    E (D |B ): p2h = p1+sv,        det = z2-dd
    F (E'|- ): rr = clamp(det*e1), r2 = rr^2     (e1 ACT-written into E.lo)
    G (F'|- ): t2 = rr*invs, pad   (invs ACT-written into F.hi)
    Y (H |P ): m5 = P2*c1,         m7 = P2*s3
ACT (8/rep): Ln, Exp e1, Exp P2 | Ln 1-r2, Exp invs | Arctan, Sin, Sin.

Emission is software-pipelined at depth 4 (stages S0=loads+A..E+lnexp1,
S1=F+lnexp2, S2=G, S2b=trig batch x2 reps, S3=Y+store): engine queues are
in-order, so V's queue must not head-block on ACT mid-rep.  Trig ACT ops
of two reps are emitted adjacently to reduce table-set switches.

Measured (NTFF, in-NEFF unroll differencing, (T(7)-T(1))/6): 17.6us/iter
steady state (f32 stock baseline: 93.5us; bf16 stock-op baseline: 23.9us).
Engine budgets per iter: ACT 8 ops x ~2.0us + ~1.1 table loads = 17.6us
(the bottleneck, ~100% busy); Vector 8 pair passes x ~1.95us = 15.6us.
ACT strided *writes* cost +470ns each; strided reads are free.  The tile
scheduler reorders engine queues by its own topo order, so emission-order
ACT grouping does NOT survive scheduling; the act-set-blocking dep edges
below force the scheduled ACT order into 2-rep set blocks
([10 ln/exp ops][6 trig ops]), cutting table loads from ~2.9/iter to
~1.1/iter (walrus then uses exactly natural_log_exp + trig_and_small).
"""

import sys

if "/opt/trn_rl_repo" not in sys.path:
    sys.path.insert(0, "/opt/trn_rl_repo")

import math

import numpy as np


# ===== inlined pair-op machinery (hand-authored 2X_1PORT custom DVE) =====
from dataclasses import dataclass

from concourse.dve_uop import (
    DISABLE,
    ENABLE,
    AluInp,
    AluOp,
    DelayInp,
    DveOpSpec,
    InpSel,
    OutPath,
    OutSel,
    Trigger,
    UopConfig,
    UopDpConfig,
    DELAY_OUT,
)

OP_MAP = {
    "add": AluOp.ADD,
    "sub": AluOp.SUBTRACT,
    "mul": AluOp.MULTIPLY,
    "max": AluOp.MAX,
    "min": AluOp.MIN,
}

LEAF_SEL = {
    "a": InpSel.SRC_0,
    "ah": InpSel.SRC_0_HI,
    "b": InpSel.SRC_1,
    "bh": InpSel.SRC_1_HI,
    "c0": InpSel.CONST_0,
    "c1": InpSel.CONST_1,
    "c2": InpSel.CONST_2,
    "one": InpSel.ONE_F32,
    "zero": InpSel.ZERO,
}

N_BLOCKS = 8
N_CHAINS = 6


def build_pair_uop(ssa, outs):
    """Schedule `ssa` (one op per block, in order) into a UopConfig."""
    n = len(ssa)
    assert n <= N_BLOCKS, f"{n} ops > {N_BLOCKS} blocks"
    dests = [d for d, *_ in ssa]
    assert len(set(dests)) == n, "SSA dests must be unique"

    # last block that reads each value / leaf; outputs read at block 8
    last_read = {}
    for k, (_, _, sa, sb) in enumerate(ssa):
        for s in (sa, sb):
            last_read[s] = k
    born = {d: k for k, (d, *_r) in enumerate(ssa)}
    for o in outs:
        assert o in born, f"output {o} not produced"
        last_read[o] = N_BLOCKS

    # leaves used, in first-use order
    leaves = []
    for _, _, sa, sb in ssa:
        for s in (sa, sb):
            if s in LEAF_SEL and s not in leaves:
                leaves.append(s)
    assert len(leaves) <= N_CHAINS, f"too many leaves: {leaves}"

    u = UopConfig()
    u.trigger = (Trigger.SRC_TENSOR_DONE, Trigger.NONE, Trigger.NONE)
    u.next_uop = (0, 0, 0)
    u.require_inp0 = ENABLE
    uses_src1 = any(s in ("b", "bh") for s in leaves)
    u.require_inp1 = ENABLE if uses_src1 else DISABLE

    # chain c <- inp lane c+1
    loc = {}          # value/leaf -> ('chain', c) or ('alu', born_block)
    chain_free = list(range(N_CHAINS))
    chain_owner = {}  # c -> value (for pass-through bookkeeping)
    for lf in leaves:
        c = chain_free.pop(0)
        u.enable_input(LEAF_SEL[lf], c + 1)
        loc[lf] = ("chain", c)
        chain_owner[c] = lf

    # the single value allowed to ride the tail ALU-bypass chain to blk7:
    # an output born at the last block
    alu_rider = None
    for o in outs:
        if born[o] == n - 1:
            alu_rider = o
            break

    val_chain = {}  # value -> chain id (captured at born+1, visible born+2)

    def op_src(s, k):
        where = loc[s]
        if where[0] == "chain":
            return AluInp(AluInp.PREV_DELAY_0 + where[1])
        if where == ("alu", k - 1):
            return AluInp.PREV_ALU_OUT
        if s in val_chain and k >= where[1] + 2:
            return AluInp(AluInp.PREV_DELAY_0 + val_chain[s])
        raise AssertionError(
            f"value {s} born block {where[1]} not reachable at block {k}"
        )

    pending_capture = None  # (value, chain) to capture at current block
    for k, (d, opn, sa, sb) in enumerate(ssa):
        blk = u.datapath_config[k]
        blk.enable_alu(OP_MAP[opn], op_src(sa, k), op_src(sb, k))

        # pass-through all live chains; apply pending capture
        for c, owner in list(chain_owner.items()):
            if pending_capture and pending_capture[1] == c:
                continue
            if last_read.get(owner, -1) > k:
                blk.pass_through_delay(c)
            else:
                del chain_owner[c]
                chain_free.append(c)
                chain_free.sort()
        if pending_capture:
            v, c = pending_capture
            blk.enable_delay_from_src(DelayInp.PREV_ALU_OUT, c)
            pending_capture = None

        # where does d live?
        needs_chain = (
            last_read.get(d, -1) > k + 1
            or (d in outs and d is not alu_rider)
            or (last_read.get(d, -1) == k + 1 and d in outs and d is not alu_rider)
        )
        loc[d] = ("alu", k)
        if needs_chain:
            assert chain_free, f"no free delay chain for {d} at block {k}"
            c = chain_free.pop(0)
            chain_owner[c] = d
            pending_capture = (d, c)
            val_chain[d] = c
            # consumers at k+1 still read PREV_ALU_OUT; from k+2 on, the
            # chain (op_src falls back through val_chain).

    # tail blocks: bypass ALU (carries last op's result), pass chains through
    for k in range(n, N_BLOCKS):
        blk = u.datapath_config[k]
        blk.pass_through_alu()
        for c, owner in list(chain_owner.items()):
            if pending_capture and pending_capture[1] == c:
                continue
            if last_read.get(owner, -1) > k:
                blk.pass_through_delay(c)
        if pending_capture:
            v, c = pending_capture
            blk.enable_delay_from_src(DelayInp.PREV_ALU_OUT, c)
            pending_capture = None
        # values captured become chain-resident
        for cc, owner in chain_owner.items():
            loc[owner] = ("chain", cc)

    # resolve chain locations for captured values (loc may still say 'alu')
    for c, owner in chain_owner.items():
        loc[owner] = ("chain", c)

    # outputs
    def out_sel(o):
        if o is alu_rider:
            return OutSel.ALU_OUT
        where = loc[o]
        assert where[0] == "chain", f"output {o} lost: {where}"
        return DELAY_OUT[where[1]]

    if len(outs) == 2:
        u.enable_output(out_sel(outs[0]), OutPath.WR0_LO)
        u.enable_output(out_sel(outs[1]), OutPath.WR0_HI)
    else:
        u.enable_output(out_sel(outs[0]), OutPath.WR0_LO)
    u.validate("v3")
    return u


@dataclass
class PairOp:
    name: str
    ssa: list
    outs: tuple
    uses_src1: bool


_REGISTERED = {}


def register_pair_op(name, ssa, outs):
    """Build + register a pair op under dve_ops so the per-NEFF table
    includes it. Returns the DveOp for _custom_dve."""
    from concourse import dve_ops
    from concourse.dve_spec import Spec, Src0, Src1, C0, C1, C2, sq

    if name in _REGISTERED:
        return _REGISTERED[name]

    uop = build_pair_uop(ssa, outs)
    leaves = {s for _, _, sa, sb in ssa for s in (sa, sb)}
    # ALWAYS two-stream: a single-src packed-bf16 op would be auto-promoted
    # to 2x_2p/4x mode slots whose data rates don't match our 2x program
    # (observed NRT_EXEC_UNIT_UNRECOVERABLE). With rd1_en the engine treats
    # it as TT-class and caps detection at 2X_1PORT. Callers pass a dummy
    # in1 when the program doesn't read b/bh.
    uses_src1 = True
    uop.require_inp1 = ENABLE

    # dummy Spec carrying the right leaf set for _custom_dve's asserts
    body = Src0 + Src0
    if uses_src1:
        body = body + Src1
    if "c0" in leaves:
        body = body * C0
    if "c1" in leaves:
        body = body * C1
    if "c2" in leaves:
        body = body * C2
    spec = Spec(body=body, reference=lambda *a, **k: None)

    if name in dve_ops._SUB_OPCODE_FOR_NAME:
        row = dve_ops._SUB_OPCODE_FOR_NAME[name]
        op = dve_ops.DveOp(name, spec, subdim=False, uops_sha={})
        dve_ops.OPS[:] = [o for o in dve_ops.OPS if o.name != name] + [op]
    else:
        row = max(dve_ops._SUB_OPCODE_FOR_NAME.values()) + 1
        assert row < 0x20, "byte-36 row overflow"
        op = dve_ops.DveOp(name, spec, subdim=False, uops_sha={})
        dve_ops.OPS.append(op)
        dve_ops._SUB_OPCODE_FOR_NAME[name] = row
    dve_ops.CUSTOM_DVE_SPECS[name] = spec

    compiled = DveOpSpec(
        name=name,
        opcode=row,
        uops=[uop],      # 1x slot: same program; 1x must never engage
        uops_2x=[uop],
        perf_max=1,
        rd1_en=uses_src1,
    )
    dve_ops._COMPILE_CACHE[(name, "v3")] = compiled
    _REGISTERED[name] = op
    return op


def emit(nc, op, out, in0, in1=None, s0=0.0, s1=0.0, imm2=0.0):
    """Emit one pair-op instruction with perf_max=1 (2x_1p reachable)."""
    for ap in (out, in0) + ((in1,) if in1 is not None else ()):
        pass
    bi = nc.vector._custom_dve(
        op, out=out, in0=in0, in1=in1, s0=s0, s1=s1, imm2=imm2
    )
    bi.ins.perf_max = 1
    return bi


def eval_ssa(ssa, outs, env):
    """numpy reference for a pair program; env maps leaves to arrays."""
    import numpy as np

    vals = dict(env)
    vals.setdefault("one", 1.0)
    vals.setdefault("zero", 0.0)
    fns = {
        "add": lambda x, y: x + y,
        "sub": lambda x, y: x - y,
        "mul": lambda x, y: x * y,
        "max": lambda x, y: __import__("numpy").maximum(x, y),
        "min": lambda x, y: __import__("numpy").minimum(x, y),
    }
    for d, opn, sa, sb in ssa:
        vals[d] = fns[opn](vals[sa], vals[sb])
    return [vals[o] for o in outs]


# ===== kernel =====


N_CORES = 8
B = 2
DHW = 96 * 96 * 96
PER = DHW // N_CORES        # 110592 voxels per batch per core
P = 128
FB = PER // P               # 864
NP = B * FB                 # 1728 voxel-pairs per partition per core
FD = 2 * NP                 # 3456 bf16 elems per pair tile row

EPS_P2 = 1e-8
R2C = 1.0 - 2.0 ** -8
CL = math.sqrt(R2C)
B1 = 1.5 * math.log(6.0) - 2.5 * math.log(2.0)
B2 = 1.5 * math.log(2.0) - 0.5 * math.log(6.0)
TWO_PI_3 = 2.0 * math.pi / 3.0

_CACHE = {}

OPA = register_pair_op(
    "EIGA",
    [
        ("m3", "mul", "a", "ah"),
        ("m4", "mul", "m3", "b"),
        ("u2", "mul", "bh", "bh"),
        ("d3", "mul", "u2", "b"),
        ("z", "sub", "m4", "d3"),
        ("s2", "sub", "u2", "m3"),
    ],
    ("s2", "z"),
)
OPB = register_pair_op(
    "EIGB",
    [
        ("v2", "mul", "a", "a"),
        ("w2", "mul", "ah", "ah"),
        ("sv", "add", "v2", "w2"),
        ("d1", "mul", "b", "w2"),
        ("d2", "mul", "bh", "v2"),
        ("dd", "add", "d1", "d2"),
    ],
    ("sv", "dd"),
)
OPC = register_pair_op(
    "EIGC",
    [
        ("mv", "mul", "a", "ah"),
        ("mu", "mul", "mv", "bh"),
        ("m2", "mul", "mu", "c0"),
        ("c2", "mul", "b", "b"),
    ],
    ("m2", "c2"),
)
OPD = register_pair_op(
    "EIGD",
    [("p1", "add", "a", "bh"), ("z2", "add", "ah", "b")],
    ("p1", "z2"),
)
OPE = register_pair_op(
    "EIGE",
    [("p2h", "add", "a", "b"), ("det", "sub", "ah", "bh")],
    ("p2h", "det"),
)
OPF = register_pair_op(
    "EIGF",
    [
        ("t", "mul", "a", "ah"),
        ("tc", "min", "t", "c0"),
        ("rr", "max", "tc", "c1"),
        ("r2", "mul", "rr", "rr"),
    ],
    ("rr", "r2"),
)
OPG = register_pair_op(
    "EIGG",
    [("t2", "mul", "a", "ah"), ("j", "add", "t2", "zero")],
    ("t2", "j"),
)
OPY = register_pair_op(
    "EIGYM",
    [("m5", "mul", "bh", "a"), ("m7", "mul", "bh", "ah")],
    ("m5", "m7"),
)


def _build(nrep=1, split_waits=True):
    import concourse.bass as bass
    import concourse.tile as tile
    from concourse import mybir

    fp16 = mybir.dt.bfloat16
    fp32 = mybir.dt.float32
    AF = mybir.ActivationFunctionType

    nc = bass.Bass("TRN2", target_bir_lowering=False, debug=False,
                   num_devices=N_CORES)
    x = nc.dram_tensor("x", [3, P, FD], fp16, kind="ExternalInput").ap()
    y = nc.dram_tensor("y", [1, P, FD], fp16, kind="ExternalOutput").ap()

    for cval in (B1, B2, TWO_PI_3, EPS_P2, 1.0):
        cval = float(cval)
        if (fp32, cval) not in nc.const_aps.aps:
            ctens = nc.alloc_sbuf_tensor(f"const-f32-{cval}", [128, 1], fp32)
            nc.gpsimd.memset(ctens.ap(), cval)
            nc.const_aps.aps[(fp32, cval)] = ctens.ap()
    nc.all_engine_barrier()

    S = nc.scalar
    NPAR = 4
    # slot aliasing (live ranges don't overlap within a parity):
    SLOT = {"pab": 0, "E": 0, "pcu": 1, "F": 1, "pvw": 2, "D": 2, "H": 2,
            "C": 3, "tP": 3, "A": 4, "Y": 4, "B": 5, "G": 5,
            "LN": 6, "LM": 6, "AT": 6}

    from concourse.bass import _add_dep_helper
    act_h = {}

    with tile.TileContext(nc) as tc:
        with tc.tile_pool(name="eig", bufs=1) as pool:
            tiles = [{} for _ in range(nrep)]

            def T(rep, name, dtype=fp16, w=FD):
                par = rep % NPAR
                t = pool.tile([P, w], dtype, tag=f"s{SLOT[name]}p{par}",
                              name=f"{name}r{rep}")
                tiles[rep][name] = t
                return t

            def stage0(r):
                pab, pcu, pvw = (T(r, n) for n in ("pab", "pcu", "pvw"))
                for t, ch in ((pab, 0), (pcu, 1), (pvw, 2)):
                    nc.sync.dma_start(out=t[:, :], in_=x[ch][:, :])
                tA, tB, tC = T(r, "A"), T(r, "B"), T(r, "C")
                emit(nc, OPA, out=tA[:, :], in0=pab[:, :],
                             in1=pcu[:, :])
                emit(nc, OPB, out=tB[:, :], in0=pvw[:, :],
                             in1=pab[:, :])
                emit(nc, OPC, out=tC[:, :], in0=pvw[:, :],
                             in1=pcu[:, :], s0=2.0)
                tD, tE = T(r, "D"), T(r, "E")
                emit(nc, OPD, out=tD[:, :], in0=tA[:, :],
                             in1=tC[:, :])
                emit(nc, OPE, out=tE[:, :], in0=tD[:, :],
                             in1=tB[:, :])
                LN = T(r, "LN", fp32, NP)
                act_h[(r, "lnp2")] = S.activation(LN[:, :], tE[:, 0::2],
                                                  AF.Ln, bias=float(EPS_P2))
                S.activation(tE[:, 0::2], LN[:, :], AF.Exp, scale=-1.5,
                             bias=float(B1))
                tP = T(r, "tP")
                act_h[(r, "P2")] = S.activation(tP[:, 1::2], LN[:, :],
                                                AF.Exp, scale=0.5,
                                                bias=float(B2))

            def stage1(r):
                tE = tiles[r]["E"]
                tF = T(r, "F")
                emit(nc, OPF, out=tF[:, :], in0=tE[:, :],
                             in1=tE[:, :], s0=CL, s1=-CL)
                LM = T(r, "LM", fp32, NP)
                S.activation(LM[:, :], tF[:, 1::2], AF.Ln, scale=-1.0,
                             bias=1.0)
                act_h[(r, "invs")] = S.activation(tF[:, 1::2], LM[:, :],
                                                  AF.Exp, scale=-0.5)

            def stage2v(r):
                tF = tiles[r]["F"]
                tG = T(r, "G")
                emit(nc, OPG, out=tG[:, :], in0=tF[:, :],
                             in1=tF[:, :])

            def stage2a(r):
                tG = tiles[r]["G"]
                AT = T(r, "AT", fp32, NP)
                act_h[(r, "at")] = S.activation(AT[:, :], tG[:, 0::2],
                                                AF.Arctan)
                tH = T(r, "H")
                act_h[(r, "c1")] = S.activation(tH[:, 0::2], AT[:, :], AF.Sin,
                                                scale=-1.0 / 3.0,
                                                bias=float(TWO_PI_3))
                act_h[(r, "s3")] = S.activation(tH[:, 1::2], AT[:, :], AF.Sin,
                                                scale=1.0 / 3.0)

            def stage3(r):
                tH, tP = tiles[r]["H"], tiles[r]["tP"]
                tY = T(r, "Y")
                emit(nc, OPY, out=tY[:, :], in0=tH[:, :],
                             in1=tP[:, :])
                nc.sync.dma_start(out=y[0][:, :], in_=tY[:, :])

            for it in range(nrep + 4):
                if it < nrep:
                    stage0(it)
                if 0 <= it - 1 < nrep:
                    stage1(it - 1)
                if 0 <= it - 2 < nrep:
                    stage2v(it - 2)
                # batch two reps' trig ACT ops adjacently: one table-set
                # switch per rep instead of two.
                if it % 2 == 1 or it == nrep + 3:
                    for j in (it - 3, it - 2):
                        if 0 <= j < nrep and "AT" not in tiles[j]:
                            stage2a(j)
                    for j in (it - 3, it - 2):
                        if 0 <= j < nrep and "Y" not in tiles[j]:
                            stage3(j)

            # The tile scheduler reorders engine queues by its own topo
            # order, scrambling emission-order set grouping (measured: trig
            # ops of old reps interleave lnexp runs -> ~2.9 table loads per
            # rep). Force 2-rep set blocks with scheduling deps:
            #   trig(g) after every lnexp terminal of group g = (r, r+1);
            #   lnexp of the NEXT group after trig terminals of group g.
            def dep(a, b):
                if a is not None and b is not None:
                    _add_dep_helper(a.ins, b.ins, sync=False,
                                    reason="act-set-blocking")

            for r0 in range(0, nrep, 2):
                grp = [r for r in (r0, r0 + 1) if r < nrep]
                first_trig = act_h.get((r0, "at"))
                for r in grp:
                    for t in ("invs", "P2"):
                        dep(first_trig, act_h.get((r, t)))
                nxt = [r for r in (r0 + 2, r0 + 3) if r < nrep]
                for rn in nxt:
                    for r in grp:
                        for t in ("c1", "s3"):
                            dep(act_h.get((rn, "lnp2")), act_h.get((r, t)))

    from concourse.library_overlay import lower_extended_insts
    lower_extended_insts(nc)
    if split_waits:
        _split_multi_waits(nc, mybir)
    return nc


def _split_multi_waits(nc, mybir):
    """walrus codegen allows a single sync-wait slot per TPB instruction;
    hoist extra waits onto standalone NoOps on the same engine."""
    for f in nc.m.functions:
        for blk in f.blocks:
            il = blk.instructions
            i = 0
            while i < len(il):
                inst = il[i]
                si = inst.sync_info
                if si is not None and si.on_wait and len(si.on_wait) > 1:
                    waits = list(si.on_wait)
                    for w in waits[:-1]:
                        nop = mybir.InstNoOp(
                            name=nc.get_next_instruction_name(),
                            engine=inst.engine,
                            ins=[],
                            outs=[],
                            sync_info=mybir.SyncInfo(on_wait=[w], on_update=[]),
                            bass_nofuse=True,
                        )
                        il.insert(i, nop)
                        i += 1
                    si.on_wait = waits[-1:]
                i += 1


def get_program():
    if "nc" not in _CACHE:
        _CACHE["nc"] = _build()
    return _CACHE["nc"]


def shard_inputs(X):
    """X: (2,9,96,96,96) f32 -> per-core {"x": (3,P,FD) bf16} pair channels
    (a,b), (c,u), (v,w), plus host-side q per core for unshard."""
    import ml_dtypes

    x = np.asarray(X, dtype=np.float32).reshape(B, 9, DHW)
    q = (x[:, 0] + x[:, 4] + x[:, 8]) * (1.0 / 3.0)
    ch = np.empty((6, B, DHW), dtype=np.float32)
    ch[0] = x[:, 0] - q
    ch[1] = x[:, 4] - q
    ch[2] = x[:, 8] - q
    ch[3] = (x[:, 1] + x[:, 3]) * 0.5
    ch[4] = (x[:, 2] + x[:, 6]) * 0.5
    ch[5] = (x[:, 5] + x[:, 7]) * 0.5
    pairs = [(0, 1), (2, 3), (4, 5)]
    maps = []
    for c in range(N_CORES):
        slab = ch[:, :, c * PER:(c + 1) * PER].reshape(6, B, P, FB)
        slab = np.ascontiguousarray(slab.transpose(0, 2, 1, 3)).reshape(6, P, NP)
        xc = np.empty((3, P, FD), dtype=ml_dtypes.bfloat16)
        for k, (lo, hi) in enumerate(pairs):
            xc[k, :, 0::2] = slab[lo]
            xc[k, :, 1::2] = slab[hi]
        maps.append({"x": xc})
    return maps, q


def unshard_outputs(results, q):
    out = np.empty((B, 3, DHW), dtype=np.float32)
    for c, r in enumerate(results):
        yc = np.asarray(r["y"]).astype(np.float32)   # (1, P, FD)
        m5 = yc[0, :, 0::2]
        m7 = yc[0, :, 1::2]
        # (P, NP) -> (B, PER)
        m5 = m5.reshape(P, B, FB).transpose(1, 0, 2).reshape(B, PER)
        m7 = m7.reshape(P, B, FB).transpose(1, 0, 2).reshape(B, PER)
        qc = q[:, c * PER:(c + 1) * PER]
        out[:, 0, c * PER:(c + 1) * PER] = qc - m5 + m7   # lmin
        out[:, 1, c * PER:(c + 1) * PER] = qc - m7        # lmid
        out[:, 2, c * PER:(c + 1) * PER] = qc + m5        # lmax
    return out.reshape(B, 3, 96, 96, 96)


def kernel(X):
    from concourse.bass_utils import run_bass_kernel_spmd

    nc = get_program()
    in_maps, q = shard_inputs(np.asarray(X))
    res = run_bass_kernel_spmd(nc, in_maps, list(range(N_CORES)))
    return unshard_outputs(res.results, q)
